# revision 6
# baseline (speedup 1.0000x reference)
"""Trainium2 Bass kernel for nn_DiffMultiHeadAttention.

Sharding: tensor-parallel over heads — 16 heads / 8 cores = 2 heads per core.
Each core computes its 2 heads' QKV projections, scores, softmax, A (output),
A@V, per-head LayerNorm, and a partial WO product. Host sums the partial WO
outputs (+bias) and concatenates the A shards.

Device math notes:
 - scale 1/sqrt(1024)=1/32 folded into wq/bq on host.
 - mask handled via an augmented 65th contraction row: Q' gets a ones row,
   K' gets a row of (mask-1)*1e30, so scores = Q'.K + maskbias.
 - softmax without max-subtraction (scores are O(1) by construction; exact
   same math, fp32-safe).
 - softmax denominator l comes from a ones-column appended to V: the AV
   matmul computes [P@V | rowsum(P)] in one accumulation group.
 - LN gamma/beta (and the 1-LAMBDA_INIT factor) folded into WO + an output
   offset vector on host; device LN is just (x-mu)*rsqrt(var+eps).
"""
import os
import sys
import types

import numpy as np

H, D, DI, B, S = 16, 1024, 64, 2, 2048
HPC = 2            # heads per core
NCORES = 8
P = 128
KC = D // P        # 8 contraction chunks over d_model
NT = S // P        # 16 query tiles
TC = S // P        # 16 key/t chunks
LN_EPS = 1e-5
LAMBDA_INIT = 0.8
SCALE = 1.0 / 32.0


def _setup_paths():
    for p in ("/opt/trn_rl_repo", os.path.expanduser("~/.axon_site/_ro/trn_rl_repo")):
        if os.path.isdir(p) and p not in sys.path:
            sys.path.append(p)


def _setup_ntff_hook():
    """Register the axon NTFF profile hook if the antenv stub lacks it."""
    try:
        import antenv
        if "antenv.axon_hooks" in sys.modules:
            return
        holder = [None]
        mod = types.ModuleType("antenv.axon_hooks")
        mod.set_axon_ntff_profile_hook = lambda h: holder.__setitem__(0, h)
        mod.get_axon_ntff_profile_hook = lambda: holder[0]
        sys.modules["antenv.axon_hooks"] = mod
        antenv.axon_hooks = mod
        from trn_agent_boot.trn_boot import _ntff_profile_via_ctypes
        hook = _ntff_profile_via_ctypes('/opt/axon/libaxon_pjrt.so')
        if hook is not None:
            mod.set_axon_ntff_profile_hook(hook)
    except Exception:
        pass


_CACHE = {}


def _build_program():
    import concourse.bacc as bacc
    import concourse.tile as tile
    from concourse import mybir
    from concourse.masks import make_identity

    fp32 = mybir.dt.float32
    AF = mybir.ActivationFunctionType
    ALU = mybir.AluOpType

    nc = bacc.Bacc()
    xT = nc.declare_dram_parameter("xT", [B, D, S], fp32, isOutput=False)
    wqk = nc.declare_dram_parameter("wqk", [HPC, D, P], fp32, isOutput=False)
    wv = nc.declare_dram_parameter("wv", [HPC, D, P], fp32, isOutput=False)
    bqk = nc.declare_dram_parameter("bqk", [HPC, P], fp32, isOutput=False)
    bvr = nc.declare_dram_parameter("bvr", [P, HPC, P], fp32, isOutput=False)
    wo = nc.declare_dram_parameter("wo", [HPC, P, D], fp32, isOutput=False)
    mb = nc.declare_dram_parameter("mb", [B + 1, S], fp32, isOutput=False)
    oml = nc.declare_dram_parameter("oml", [P, 1], fp32, isOutput=False)
    a_out = nc.declare_dram_parameter("a_out", [HPC, B, S, S], fp32, isOutput=True)
    res_out = nc.declare_dram_parameter("res", [B, S, D], fp32, isOutput=True)

    from contextlib import ExitStack

    with ExitStack() as es:
        tc = es.enter_context(tile.TileContext(nc))
        const = es.enter_context(tc.tile_pool(name="const", bufs=1))
        xt_pool = es.enter_context(tc.tile_pool(name="xt", bufs=1))
        qt_pool = es.enter_context(tc.tile_pool(name="qt", bufs=1))
        kt_pool = es.enter_context(tc.tile_pool(name="kt", bufs=1))
        v_pool = es.enter_context(tc.tile_pool(name="vp", bufs=2))
        vt_pool = es.enter_context(tc.tile_pool(name="vt", bufs=2))
        p_pool = es.enter_context(tc.tile_pool(name="pp", bufs=2))
        pt_pool = es.enter_context(tc.tile_pool(name="pt", bufs=3))
        a_pool = es.enter_context(tc.tile_pool(name="aa", bufs=2))
        av_pool = es.enter_context(tc.tile_pool(name="av", bufs=4))
        ln_pool = es.enter_context(tc.tile_pool(name="ln", bufs=4))
        ot_pool = es.enter_context(tc.tile_pool(name="ot", bufs=2))
        r_pool = es.enter_context(tc.tile_pool(name="rr", bufs=2))
        acc_pool = es.enter_context(tc.tile_pool(name="acc", bufs=2))
        ps_sn = es.enter_context(tc.tile_pool(name="ps_sn", bufs=1, space="PSUM"))
        ps_st = es.enter_context(tc.tile_pool(name="ps_st", bufs=2, space="PSUM"))
        ps_av = es.enter_context(tc.tile_pool(name="ps_av", bufs=2, space="PSUM"))
        ps_tr = es.enter_context(tc.tile_pool(name="ps_tr", bufs=2, space="PSUM"))
        if True:
            fr = mybir.dt.float32r
            ident = const.tile([P, P], fp32)
            make_identity(nc, ident)
            wqk_sb = const.tile([P, HPC, KC, P], fr)
            nc.gpsimd.dma_start(
                out=wqk_sb,
                in_=wqk[:].rearrange("j (k p) m -> p j k m", p=P).bitcast(fr))
            wv_sb = const.tile([P, HPC, KC, P], fr)
            nc.gpsimd.dma_start(
                out=wv_sb,
                in_=wv[:].rearrange("j (k p) m -> p j k m", p=P).bitcast(fr))
            wo_sb = const.tile([P, HPC, D], fr)
            nc.gpsimd.dma_start(
                out=wo_sb, in_=wo[:].rearrange("j p m -> p j m").bitcast(fr))
            bqk_sb = const.tile([P, HPC], fp32)
            nc.gpsimd.dma_start(out=bqk_sb, in_=bqk[:].rearrange("j p -> p j"))
            oml_sb = const.tile([P, 1], fp32)
            nc.gpsimd.dma_start(out=oml_sb, in_=oml[:])
            eps_sb = const.tile([P, 1], fp32)
            nc.vector.memset(eps_sb, LN_EPS)
            bvr_sb = const.tile([P, HPC, P], fp32)
            nc.gpsimd.dma_start(out=bvr_sb, in_=bvr[:])
            bvo_sb = const.tile([P, HPC, P], fp32)
            # (1-lambda)*bv, replicated on all partitions
            nc.vector.tensor_scalar_mul(bvo_sb, bvr_sb, oml_sb)

            for b in range(B):
                xt = xt_pool.tile([P, KC, S], fr, tag="xt")
                nc.sync.dma_start(
                    out=xt, in_=xT[b].rearrange("(k p) s -> p k s", p=P).bitcast(fr))
                ots = []
                for j in range(HPC):
                    # ---- QK projection: psum rows 0:64 = Q-scaled, 64:128 = K
                    qt = qt_pool.tile([P, S], fr, tag="qt")
                    kt = kt_pool.tile([P, S], fr, tag="kt")
                    nc.sync.dma_start(out=qt[64:65, :], in_=mb[B:B + 1, :].bitcast(fr))
                    nc.sync.dma_start(out=kt[64:65, :], in_=mb[b:b + 1, :].bitcast(fr))
                    for n in range(S // 512):
                        ps = ps_tr.tile([P, 512], fp32, tag="ptr")
                        for k in range(KC):
                            nc.tensor.matmul(
                                ps, wqk_sb[:, j, k, :],
                                xt[:, k, n * 512:(n + 1) * 512],
                                start=(k == 0), stop=(k == KC - 1))
                        nc.vector.tensor_scalar_add(
                            qt[0:64, n * 512:(n + 1) * 512], ps[0:64, :],
                            bqk_sb[0:64, j:j + 1])
                        nc.vector.tensor_scalar_add(
                            kt[0:64, n * 512:(n + 1) * 512], ps[64:128, :],
                            bqk_sb[64:128, j:j + 1])
                    # ---- V projection (as V^T, then transpose to [t, v]) ----
                    vsb = v_pool.tile([P, TC, P], fr, tag="v")
                    for n in range(S // 512):
                        psv = ps_tr.tile([P, 512], fp32, tag="ptr")
                        for k in range(KC):
                            nc.tensor.matmul(
                                psv, wv_sb[:, j, k, :],
                                xt[:, k, n * 512:(n + 1) * 512],
                                start=(k == 0), stop=(k == KC - 1))
                        vts = vt_pool.tile([P, 512], fp32, tag="vt")
                        nc.vector.tensor_copy(vts, psv)
                        pst = ps_tr.tile([P, 512], fp32, tag="ptr")
                        for c4 in range(4):
                            nc.tensor.transpose(
                                pst[:, c4 * P:(c4 + 1) * P],
                                vts[:, c4 * P:(c4 + 1) * P], ident)
                        nc.vector.tensor_copy(vsb[:, n * 4:(n + 1) * 4, :], pst)
                    # ---- attention: phases interleaved per 512-wide sq block
                    cv_all = acc_pool.tile([P, NT], fp32, tag="cva")
                    mv_all = acc_pool.tile([P, NT, 2], fp32, tag="mva")
                    av_all = acc_pool.tile([P, NT, P], fp32, tag="ava")
                    ot = ot_pool.tile([P, NT, P], fr, tag="ot")
                    for beta in range(4):
                        sq0 = beta * 512
                        # phase A: natural scores -> exp(+rowsum) -> A
                        for n in range(beta * 4, beta * 4 + 4):
                            pex = p_pool.tile([P, S], fp32, tag="pp")
                            lh = ln_pool.tile([P, 2], fp32, tag="lh")
                            for h2 in range(2):
                                psn = ps_sn.tile([P, 1024], fp32, tag="sn")
                                for c in range(2):
                                    nc.tensor.matmul(
                                        psn[:, c * 512:(c + 1) * 512],
                                        qt[0:65, n * P:(n + 1) * P],
                                        kt[0:65, h2 * 1024 + c * 512:
                                           h2 * 1024 + (c + 1) * 512],
                                        start=True, stop=True)
                                nc.scalar.activation(
                                    pex[:, h2 * 1024:(h2 + 1) * 1024], psn,
                                    AF.Exp, accum_out=lh[:, h2:h2 + 1])
                            lsum = ln_pool.tile([P, 1], fp32, tag="ls")
                            nc.vector.tensor_add(lsum, lh[:, 0:1], lh[:, 1:2])
                            linv = ln_pool.tile([P, 1], fp32, tag="li")
                            nc.vector.reciprocal(linv, lsum)
                            nc.vector.tensor_mul(
                                cv_all[:, n:n + 1], linv, oml_sb)
                            for h2 in range(2):
                                a_sb = a_pool.tile([P, 1024], fp32, tag="aa")
                                nc.gpsimd.tensor_scalar_mul(
                                    a_sb, pex[:, h2 * 1024:(h2 + 1) * 1024],
                                    cv_all[:, n:n + 1])
                                nc.sync.dma_start(
                                    out=a_out[j, b, n * P:(n + 1) * P,
                                              h2 * 1024:(h2 + 1) * 1024],
                                    in_=a_sb)
                        # phase B: transposed scores -> exp -> AV^T
                        ps_o = ps_av.tile([P, 512], fp32, tag="av")
                        for c in range(TC):
                            ps_t = ps_st.tile([P, 512], fp32, tag="st")
                            nc.tensor.matmul(
                                ps_t, kt[0:65, c * P:(c + 1) * P],
                                qt[0:65, sq0:sq0 + 512],
                                start=True, stop=True)
                            ptc = pt_pool.tile([P, 512], fr, tag="pt")
                            nc.scalar.activation(ptc, ps_t, AF.Exp)
                            nc.tensor.matmul(
                                ps_o, vsb[:, c, :], ptc,
                                start=(c == 0), stop=(c == TC - 1))
                        avt = vt_pool.tile([P, 512], fp32, tag="avt")
                        nc.vector.tensor_copy(avt, ps_o)
                        for q4 in range(4):
                            n = beta * 4 + q4
                            pstq = ps_tr.tile([P, 512], fp32, tag="ptr")
                            nc.tensor.transpose(
                                pstq[:, 0:P], avt[:, q4 * P:(q4 + 1) * P],
                                ident)
                            av = av_all[:, n, :]
                            nc.vector.tensor_scalar_mul(
                                av, pstq[:, 0:P], cv_all[:, n:n + 1])
                            nc.vector.tensor_add(av, av, bvo_sb[:, j, :])
                            stats = ln_pool.tile([P, 6], fp32, tag="st6")
                            nc.vector.bn_stats(stats, av)
                            nc.vector.bn_aggr(mv_all[:, n, :], stats)
                    # ---- batched LN rstd, apply, transpose to ot ----
                    rstd = ln_pool.tile([P, NT], fp32, tag="rsd")
                    nc.scalar.activation(
                        rstd, mv_all[:, :, 1], AF.Sqrt, bias=eps_sb)
                    nc.vector.reciprocal(rstd, rstd)
                    for n in range(NT):
                        lno = av_pool.tile([P, P], fp32, tag="lno")
                        nc.vector.tensor_scalar(
                            lno, av_all[:, n, :], mv_all[:, n, 0:1],
                            rstd[:, n:n + 1],
                            op0=ALU.subtract, op1=ALU.mult)
                        psl = ps_tr.tile([P, 512], fp32, tag="ptr")
                        nc.tensor.transpose(psl[:, 0:P], lno, ident)
                        nc.vector.tensor_copy(ot[:, n, :], psl[:, 0:P])
                    ots.append(ot)
                # ---- partial WO for this batch ----
                for n in range(NT):
                    for half in range(2):
                        psw = ps_tr.tile([P, 512], fp32, tag="ptr")
                        for j in range(HPC):
                            nc.tensor.matmul(
                                psw, ots[j][:, n, :],
                                wo_sb[:, j, half * 512:(half + 1) * 512],
                                start=(j == 0), stop=(j == HPC - 1))
                        rsb = r_pool.tile([P, 512], fp32, tag="rr")
                        nc.vector.tensor_copy(rsb, psw)
                        nc.sync.dma_start(
                            out=res_out[b, n * P:(n + 1) * P,
                                        half * 512:(half + 1) * 512],
                            in_=rsb)

    nc.finalize()
    return nc


def _prep_inputs(x, lambda_current, mask, wq, bq, wk, bk, wv, bv, wo):
    """Host-side sharding/marshaling. Returns (shared, percore) dicts."""
    f = np.float32
    xT = np.ascontiguousarray(np.transpose(np.asarray(x, f), (0, 2, 1)))
    mbias = (np.asarray(mask, f) - 1.0) * np.float32(1e30)
    mbias = np.concatenate([mbias, np.ones((1, S), f)], axis=0)
    omlv = np.full((P, 1), 1.0 - float(lambda_current), f)
    shared = {"xT": xT, "mb": mbias, "oml": omlv}
    percore = []
    for i in range(NCORES):
        h0 = i * HPC
        wqk_i = np.stack([
            np.concatenate([wq[h] * SCALE, wk[h]], axis=1)
            for h in range(h0, h0 + HPC)]).astype(f)
        bqk_i = np.stack([
            np.concatenate([bq[h] * SCALE, bk[h]])
            for h in range(h0, h0 + HPC)]).astype(f)
        wv_i = np.ascontiguousarray(wv[h0:h0 + HPC], f)
        bvr_i = np.broadcast_to(
            bv[h0:h0 + HPC][None, :, :], (P, HPC, P)).astype(f)
        wo_i = np.ascontiguousarray(
            wo[h0 * P:(h0 + HPC) * P].reshape(HPC, P, D), f)
        percore.append({
            "wqk": wqk_i, "bqk": bqk_i, "wv": wv_i, "bvr": bvr_i, "wo": wo_i})
    return shared, percore


def kernel(x, lambda_current, mask, wq, bq, wk, bk, wv, bv, wo, bo, ln_g, ln_b):
    _setup_paths()
    _setup_ntff_hook()
    from concourse.bass_utils import run_bass_kernel_spmd

    f = np.float32
    x = np.asarray(x, f)
    wq = np.asarray(wq, f); bq = np.asarray(bq, f)
    wk = np.asarray(wk, f); bk = np.asarray(bk, f)
    wv_a = np.asarray(wv, f); bv_a = np.asarray(bv, f)
    wo_a = np.asarray(wo, f); bo_a = np.asarray(bo, f)
    ln_g = np.asarray(ln_g, f); ln_b = np.asarray(ln_b, f)

    # Fold LN gamma (and 1-LAMBDA_INIT) into WO; beta becomes an output offset.
    gfac = (1.0 - LAMBDA_INIT) * ln_g            # [128]
    gtile = np.tile(gfac, H)                     # [2048]
    wo_eff = (wo_a * gtile[:, None]).astype(f)
    btile = np.tile((1.0 - LAMBDA_INIT) * ln_b, H)
    out_off = (btile @ wo_a + bo_a).astype(f)    # [1024]

    shared, percore = _prep_inputs(
        x, lambda_current, mask, wq, bq, wk, bk, wv_a, bv_a, wo_eff)

    if "nc" not in _CACHE:
        _CACHE["nc"] = _build_program()
    nc = _CACHE["nc"]

    in_maps = [dict(shared, **pc) for pc in percore]
    trace = bool(os.environ.get("BASS_KERNEL_TRACE"))
    r = run_bass_kernel_spmd(nc, in_maps, list(range(NCORES)), trace=trace)
    _CACHE["last_exec_time_ns"] = r.exec_time_ns

    A = np.concatenate([r.results[i]["a_out"] for i in range(NCORES)], axis=0)
    res = np.zeros((B, S, D), f)
    for i in range(NCORES):
        res += r.results[i]["res"]
    res += out_off[None, None, :]
    return (res, A)


# revision 7
# speedup vs baseline: 3.3268x; 3.3268x over previous
"""Trainium2 Bass kernel for nn_DiffMultiHeadAttention.

Sharding: tensor-parallel over heads — 16 heads / 8 cores = 2 heads per core.
Each core computes its 2 heads' QKV projections, scores, softmax, A (output),
A@V, per-head LayerNorm, and a partial WO product. Host sums the partial WO
outputs (+bias) and concatenates the A shards.

Device math notes:
 - scale 1/sqrt(1024)=1/32 folded into wq/bq on host.
 - mask handled via an augmented 65th contraction row: Q' gets a ones row,
   K' gets a row of (mask-1)*1e30, so scores = Q'.K + maskbias.
 - softmax without max-subtraction (scores are O(1) by construction; exact
   same math, fp32-safe).
 - softmax denominator l comes from a ones-column appended to V: the AV
   matmul computes [P@V | rowsum(P)] in one accumulation group.
 - LN gamma/beta (and the 1-LAMBDA_INIT factor) folded into WO + an output
   offset vector on host; device LN is just (x-mu)*rsqrt(var+eps).
"""
import os
import sys
import types

import numpy as np

H, D, DI, B, S = 16, 1024, 64, 2, 2048
HPC = 2            # heads per core
NCORES = 8
P = 128
KC = D // P        # 8 contraction chunks over d_model
NT = S // P        # 16 query tiles
TC = S // P        # 16 key/t chunks
LN_EPS = 1e-5
LAMBDA_INIT = 0.8
SCALE = 1.0 / 32.0


def _setup_paths():
    for p in ("/opt/trn_rl_repo", os.path.expanduser("~/.axon_site/_ro/trn_rl_repo")):
        if os.path.isdir(p) and p not in sys.path:
            sys.path.append(p)


def _setup_ntff_hook():
    """Register the axon NTFF profile hook if the antenv stub lacks it."""
    try:
        import antenv
        if "antenv.axon_hooks" in sys.modules:
            return
        holder = [None]
        mod = types.ModuleType("antenv.axon_hooks")
        mod.set_axon_ntff_profile_hook = lambda h: holder.__setitem__(0, h)
        mod.get_axon_ntff_profile_hook = lambda: holder[0]
        sys.modules["antenv.axon_hooks"] = mod
        antenv.axon_hooks = mod
        from trn_agent_boot.trn_boot import _ntff_profile_via_ctypes
        hook = _ntff_profile_via_ctypes('/opt/axon/libaxon_pjrt.so')
        if hook is not None:
            mod.set_axon_ntff_profile_hook(hook)
    except Exception:
        pass


_CACHE = {}


def _build_program():
    import concourse.bacc as bacc
    import concourse.tile as tile
    from concourse import mybir
    from concourse.masks import make_identity

    fp32 = mybir.dt.float32
    AF = mybir.ActivationFunctionType
    ALU = mybir.AluOpType

    nc = bacc.Bacc()
    xT = nc.declare_dram_parameter("xT", [B, D, S], fp32, isOutput=False)
    wqk = nc.declare_dram_parameter("wqk", [HPC, D, P], fp32, isOutput=False)
    wv = nc.declare_dram_parameter("wv", [HPC, D, P], fp32, isOutput=False)
    bqk = nc.declare_dram_parameter("bqk", [HPC, P], fp32, isOutput=False)
    bvr = nc.declare_dram_parameter("bvr", [P, HPC, P], fp32, isOutput=False)
    wo = nc.declare_dram_parameter("wo", [HPC, P, D], fp32, isOutput=False)
    mb = nc.declare_dram_parameter("mb", [B + 1, S], fp32, isOutput=False)
    oml = nc.declare_dram_parameter("oml", [P, 1], fp32, isOutput=False)
    a_out = nc.declare_dram_parameter("a_out", [HPC, B, S, S], fp32, isOutput=True)
    res_out = nc.declare_dram_parameter("res", [B, S, D], fp32, isOutput=True)

    from contextlib import ExitStack

    with ExitStack() as es:
        tc = es.enter_context(tile.TileContext(nc))
        const = es.enter_context(tc.tile_pool(name="const", bufs=1))
        xt_pool = es.enter_context(tc.tile_pool(name="xt", bufs=1))
        qt_pool = es.enter_context(tc.tile_pool(name="qt", bufs=1))
        kt_pool = es.enter_context(tc.tile_pool(name="kt", bufs=1))
        v_pool = es.enter_context(tc.tile_pool(name="vp", bufs=2))
        vt_pool = es.enter_context(tc.tile_pool(name="vt", bufs=2))
        p_pool = es.enter_context(tc.tile_pool(name="pp", bufs=2))
        pt_pool = es.enter_context(tc.tile_pool(name="pt", bufs=3))
        a_pool = es.enter_context(tc.tile_pool(name="aa", bufs=2))
        av_pool = es.enter_context(tc.tile_pool(name="av", bufs=4))
        ln_pool = es.enter_context(tc.tile_pool(name="ln", bufs=4))
        ot_pool = es.enter_context(tc.tile_pool(name="ot", bufs=2))
        r_pool = es.enter_context(tc.tile_pool(name="rr", bufs=2))
        acc_pool = es.enter_context(tc.tile_pool(name="acc", bufs=2))
        ps_sn = es.enter_context(tc.tile_pool(name="ps_sn", bufs=1, space="PSUM"))
        ps_st = es.enter_context(tc.tile_pool(name="ps_st", bufs=2, space="PSUM"))
        ps_av = es.enter_context(tc.tile_pool(name="ps_av", bufs=2, space="PSUM"))
        ps_tr = es.enter_context(tc.tile_pool(name="ps_tr", bufs=2, space="PSUM"))
        if True:
            fr = mybir.dt.float32r
            ident = const.tile([P, P], fp32)
            make_identity(nc, ident)
            wqk_sb = const.tile([P, HPC, KC, P], fr)
            nc.gpsimd.dma_start(
                out=wqk_sb,
                in_=wqk[:].rearrange("j (k p) m -> p j k m", p=P).bitcast(fr))
            wv_sb = const.tile([P, HPC, KC, P], fr)
            nc.gpsimd.dma_start(
                out=wv_sb,
                in_=wv[:].rearrange("j (k p) m -> p j k m", p=P).bitcast(fr))
            wo_sb = const.tile([P, HPC, D], fr)
            nc.gpsimd.dma_start(
                out=wo_sb, in_=wo[:].rearrange("j p m -> p j m").bitcast(fr))
            bqk_sb = const.tile([P, HPC], fp32)
            nc.gpsimd.dma_start(out=bqk_sb, in_=bqk[:].rearrange("j p -> p j"))
            oml_sb = const.tile([P, 1], fp32)
            nc.gpsimd.dma_start(out=oml_sb, in_=oml[:])
            eps_sb = const.tile([P, 1], fp32)
            nc.vector.memset(eps_sb, LN_EPS)
            bvr_sb = const.tile([P, HPC, P], fp32)
            nc.gpsimd.dma_start(out=bvr_sb, in_=bvr[:])
            bvo_sb = const.tile([P, HPC, P], fp32)
            # (1-lambda)*bv, replicated on all partitions
            nc.vector.tensor_scalar_mul(bvo_sb, bvr_sb, oml_sb)

            for b in range(B):
                xt = xt_pool.tile([P, KC, S], fr, tag="xt")
                nc.sync.dma_start(
                    out=xt, in_=xT[b].rearrange("(k p) s -> p k s", p=P).bitcast(fr))
                ots = []
                for j in range(HPC):
                    # ---- QK projection: psum rows 0:64 = Q-scaled, 64:128 = K
                    qt = qt_pool.tile([P, S], fr, tag="qt")
                    kt = kt_pool.tile([P, S], fr, tag="kt")
                    nc.sync.dma_start(out=qt[64:65, :], in_=mb[B:B + 1, :].bitcast(fr))
                    nc.sync.dma_start(out=kt[64:65, :], in_=mb[b:b + 1, :].bitcast(fr))
                    for n in range(S // 512):
                        ps = ps_tr.tile([P, 512], fp32, tag="ptr")
                        for k in range(KC):
                            nc.tensor.matmul(
                                ps, wqk_sb[:, j, k, :],
                                xt[:, k, n * 512:(n + 1) * 512],
                                start=(k == 0), stop=(k == KC - 1))
                        nc.vector.tensor_scalar_add(
                            qt[0:64, n * 512:(n + 1) * 512], ps[0:64, :],
                            bqk_sb[0:64, j:j + 1])
                        nc.vector.tensor_scalar_add(
                            kt[0:64, n * 512:(n + 1) * 512], ps[64:128, :],
                            bqk_sb[64:128, j:j + 1])
                    # ---- V projection (as V^T, then transpose to [t, v]) ----
                    vsb = v_pool.tile([P, TC, P], fr, tag="v")
                    for n in range(S // 512):
                        psv = ps_tr.tile([P, 512], fp32, tag="ptr")
                        for k in range(KC):
                            nc.tensor.matmul(
                                psv, wv_sb[:, j, k, :],
                                xt[:, k, n * 512:(n + 1) * 512],
                                start=(k == 0), stop=(k == KC - 1))
                        vts = vt_pool.tile([P, 512], fp32, tag="vt")
                        nc.vector.tensor_copy(vts, psv)
                        pst = ps_tr.tile([P, 512], fp32, tag="ptr")
                        for c4 in range(4):
                            nc.tensor.transpose(
                                pst[:, c4 * P:(c4 + 1) * P],
                                vts[:, c4 * P:(c4 + 1) * P], ident)
                        nc.vector.tensor_copy(vsb[:, n * 4:(n + 1) * 4, :], pst)
                    # ---- attention: phases interleaved per 512-wide sq block
                    cv_all = acc_pool.tile([P, NT], fp32, tag="cva")
                    mv_all = acc_pool.tile([P, NT, 2], fp32, tag="mva")
                    av_all = acc_pool.tile([P, NT, P], fp32, tag="ava")
                    ot = ot_pool.tile([P, NT, P], fr, tag="ot")
                    for beta in range(4):
                        sq0 = beta * 512
                        # phase A: natural scores -> exp(+rowsum) -> A
                        for n in range(beta * 4, beta * 4 + 4):
                            pex = p_pool.tile([P, S], fp32, tag="pp")
                            lh = ln_pool.tile([P, 2], fp32, tag="lh")
                            for h2 in range(2):
                                psn = ps_sn.tile([P, 1024], fp32, tag="sn")
                                for c in range(2):
                                    nc.tensor.matmul(
                                        psn[:, c * 512:(c + 1) * 512],
                                        qt[0:65, n * P:(n + 1) * P],
                                        kt[0:65, h2 * 1024 + c * 512:
                                           h2 * 1024 + (c + 1) * 512],
                                        start=True, stop=True)
                                nc.scalar.activation(
                                    pex[:, h2 * 1024:(h2 + 1) * 1024], psn,
                                    AF.Exp, accum_out=lh[:, h2:h2 + 1])
                            lsum = ln_pool.tile([P, 1], fp32, tag="ls")
                            nc.vector.tensor_add(lsum, lh[:, 0:1], lh[:, 1:2])
                            linv = ln_pool.tile([P, 1], fp32, tag="li")
                            nc.vector.reciprocal(linv, lsum)
                            nc.vector.tensor_mul(
                                cv_all[:, n:n + 1], linv, oml_sb)
                            for h2 in range(2):
                                a_sb = a_pool.tile([P, 1024], fp32, tag="aa")
                                nc.vector.tensor_scalar_mul(
                                    a_sb, pex[:, h2 * 1024:(h2 + 1) * 1024],
                                    cv_all[:, n:n + 1])
                                nc.sync.dma_start(
                                    out=a_out[j, b, n * P:(n + 1) * P,
                                              h2 * 1024:(h2 + 1) * 1024],
                                    in_=a_sb)
                        # phase B: transposed scores -> exp -> AV^T
                        ps_o = ps_av.tile([P, 512], fp32, tag="av")
                        for c in range(TC):
                            ps_t = ps_st.tile([P, 512], fp32, tag="st")
                            nc.tensor.matmul(
                                ps_t, kt[0:65, c * P:(c + 1) * P],
                                qt[0:65, sq0:sq0 + 512],
                                start=True, stop=True)
                            ptc = pt_pool.tile([P, 512], fr, tag="pt")
                            nc.scalar.activation(ptc, ps_t, AF.Exp)
                            nc.tensor.matmul(
                                ps_o, vsb[:, c, :], ptc,
                                start=(c == 0), stop=(c == TC - 1))
                        avt = vt_pool.tile([P, 512], fp32, tag="avt")
                        nc.vector.tensor_copy(avt, ps_o)
                        for q4 in range(4):
                            n = beta * 4 + q4
                            pstq = ps_tr.tile([P, 512], fp32, tag="ptr")
                            nc.tensor.transpose(
                                pstq[:, 0:P], avt[:, q4 * P:(q4 + 1) * P],
                                ident)
                            av = av_all[:, n, :]
                            nc.vector.tensor_scalar_mul(
                                av, pstq[:, 0:P], cv_all[:, n:n + 1])
                            nc.vector.tensor_add(av, av, bvo_sb[:, j, :])
                            stats = ln_pool.tile([P, 6], fp32, tag="st6")
                            nc.vector.bn_stats(stats, av)
                            nc.vector.bn_aggr(mv_all[:, n, :], stats)
                    # ---- batched LN rstd, apply, transpose to ot ----
                    rstd = ln_pool.tile([P, NT], fp32, tag="rsd")
                    nc.scalar.activation(
                        rstd, mv_all[:, :, 1], AF.Sqrt, bias=eps_sb)
                    nc.vector.reciprocal(rstd, rstd)
                    for n in range(NT):
                        lno = av_pool.tile([P, P], fp32, tag="lno")
                        nc.vector.tensor_scalar(
                            lno, av_all[:, n, :], mv_all[:, n, 0:1],
                            rstd[:, n:n + 1],
                            op0=ALU.subtract, op1=ALU.mult)
                        psl = ps_tr.tile([P, 512], fp32, tag="ptr")
                        nc.tensor.transpose(psl[:, 0:P], lno, ident)
                        nc.vector.tensor_copy(ot[:, n, :], psl[:, 0:P])
                    ots.append(ot)
                # ---- partial WO for this batch ----
                for n in range(NT):
                    for half in range(2):
                        psw = ps_tr.tile([P, 512], fp32, tag="ptr")
                        for j in range(HPC):
                            nc.tensor.matmul(
                                psw, ots[j][:, n, :],
                                wo_sb[:, j, half * 512:(half + 1) * 512],
                                start=(j == 0), stop=(j == HPC - 1))
                        rsb = r_pool.tile([P, 512], fp32, tag="rr")
                        nc.vector.tensor_copy(rsb, psw)
                        nc.sync.dma_start(
                            out=res_out[b, n * P:(n + 1) * P,
                                        half * 512:(half + 1) * 512],
                            in_=rsb)

    nc.finalize()
    return nc


def _prep_inputs(x, lambda_current, mask, wq, bq, wk, bk, wv, bv, wo):
    """Host-side sharding/marshaling. Returns (shared, percore) dicts."""
    f = np.float32
    xT = np.ascontiguousarray(np.transpose(np.asarray(x, f), (0, 2, 1)))
    mbias = (np.asarray(mask, f) - 1.0) * np.float32(1e30)
    mbias = np.concatenate([mbias, np.ones((1, S), f)], axis=0)
    omlv = np.full((P, 1), 1.0 - float(lambda_current), f)
    shared = {"xT": xT, "mb": mbias, "oml": omlv}
    percore = []
    for i in range(NCORES):
        h0 = i * HPC
        wqk_i = np.stack([
            np.concatenate([wq[h] * SCALE, wk[h]], axis=1)
            for h in range(h0, h0 + HPC)]).astype(f)
        bqk_i = np.stack([
            np.concatenate([bq[h] * SCALE, bk[h]])
            for h in range(h0, h0 + HPC)]).astype(f)
        wv_i = np.ascontiguousarray(wv[h0:h0 + HPC], f)
        bvr_i = np.broadcast_to(
            bv[h0:h0 + HPC][None, :, :], (P, HPC, P)).astype(f)
        wo_i = np.ascontiguousarray(
            wo[h0 * P:(h0 + HPC) * P].reshape(HPC, P, D), f)
        percore.append({
            "wqk": wqk_i, "bqk": bqk_i, "wv": wv_i, "bvr": bvr_i, "wo": wo_i})
    return shared, percore


def kernel(x, lambda_current, mask, wq, bq, wk, bk, wv, bv, wo, bo, ln_g, ln_b):
    _setup_paths()
    _setup_ntff_hook()
    from concourse.bass_utils import run_bass_kernel_spmd

    f = np.float32
    x = np.asarray(x, f)
    wq = np.asarray(wq, f); bq = np.asarray(bq, f)
    wk = np.asarray(wk, f); bk = np.asarray(bk, f)
    wv_a = np.asarray(wv, f); bv_a = np.asarray(bv, f)
    wo_a = np.asarray(wo, f); bo_a = np.asarray(bo, f)
    ln_g = np.asarray(ln_g, f); ln_b = np.asarray(ln_b, f)

    # Fold LN gamma (and 1-LAMBDA_INIT) into WO; beta becomes an output offset.
    gfac = (1.0 - LAMBDA_INIT) * ln_g            # [128]
    gtile = np.tile(gfac, H)                     # [2048]
    wo_eff = (wo_a * gtile[:, None]).astype(f)
    btile = np.tile((1.0 - LAMBDA_INIT) * ln_b, H)
    out_off = (btile @ wo_a + bo_a).astype(f)    # [1024]

    shared, percore = _prep_inputs(
        x, lambda_current, mask, wq, bq, wk, bk, wv_a, bv_a, wo_eff)

    if "nc" not in _CACHE:
        _CACHE["nc"] = _build_program()
    nc = _CACHE["nc"]

    in_maps = [dict(shared, **pc) for pc in percore]
    trace = bool(os.environ.get("BASS_KERNEL_TRACE"))
    r = run_bass_kernel_spmd(nc, in_maps, list(range(NCORES)), trace=trace)
    _CACHE["last_exec_time_ns"] = r.exec_time_ns

    A = np.concatenate([r.results[i]["a_out"] for i in range(NCORES)], axis=0)
    res = np.zeros((B, S, D), f)
    for i in range(NCORES):
        res += r.results[i]["res"]
    res += out_off[None, None, :]
    return (res, A)


# revision 8
# speedup vs baseline: 3.3623x; 1.0107x over previous
"""Trainium2 Bass kernel for nn_DiffMultiHeadAttention.

Sharding: tensor-parallel over heads — 16 heads / 8 cores = 2 heads per core.
Each core computes its 2 heads' QKV projections, scores, softmax, A (output),
A@V, per-head LayerNorm, and a partial WO product. Host sums the partial WO
outputs (+bias) and concatenates the A shards.

Device math notes:
 - scale 1/sqrt(1024)=1/32 folded into wq/bq on host.
 - mask handled via an augmented 65th contraction row: Q' gets a ones row,
   K' gets a row of (mask-1)*1e30, so scores = Q'.K + maskbias.
 - softmax without max-subtraction (scores are O(1) by construction; exact
   same math, fp32-safe).
 - softmax denominator l comes from a ones-column appended to V: the AV
   matmul computes [P@V | rowsum(P)] in one accumulation group.
 - LN gamma/beta (and the 1-LAMBDA_INIT factor) folded into WO + an output
   offset vector on host; device LN is just (x-mu)*rsqrt(var+eps).
"""
import os
import sys
import types

import numpy as np

H, D, DI, B, S = 16, 1024, 64, 2, 2048
HPC = 2            # heads per core
NCORES = 8
P = 128
KC = D // P        # 8 contraction chunks over d_model
NT = S // P        # 16 query tiles
TC = S // P        # 16 key/t chunks
LN_EPS = 1e-5
LAMBDA_INIT = 0.8
SCALE = 1.0 / 32.0


def _setup_paths():
    for p in ("/opt/trn_rl_repo", os.path.expanduser("~/.axon_site/_ro/trn_rl_repo")):
        if os.path.isdir(p) and p not in sys.path:
            sys.path.append(p)


def _setup_ntff_hook():
    """Register the axon NTFF profile hook if the antenv stub lacks it."""
    try:
        import antenv
        if "antenv.axon_hooks" in sys.modules:
            return
        holder = [None]
        mod = types.ModuleType("antenv.axon_hooks")
        mod.set_axon_ntff_profile_hook = lambda h: holder.__setitem__(0, h)
        mod.get_axon_ntff_profile_hook = lambda: holder[0]
        sys.modules["antenv.axon_hooks"] = mod
        antenv.axon_hooks = mod
        from trn_agent_boot.trn_boot import _ntff_profile_via_ctypes
        hook = _ntff_profile_via_ctypes('/opt/axon/libaxon_pjrt.so')
        if hook is not None:
            mod.set_axon_ntff_profile_hook(hook)
    except Exception:
        pass


_CACHE = {}


def _build_program():
    import concourse.bacc as bacc
    import concourse.tile as tile
    from concourse import mybir
    from concourse.masks import make_identity

    fp32 = mybir.dt.float32
    AF = mybir.ActivationFunctionType
    ALU = mybir.AluOpType

    nc = bacc.Bacc()
    xT = nc.declare_dram_parameter("xT", [B, D, S], fp32, isOutput=False)
    wqk = nc.declare_dram_parameter("wqk", [HPC, D, P], fp32, isOutput=False)
    wv = nc.declare_dram_parameter("wv", [HPC, D, P], fp32, isOutput=False)
    bqk = nc.declare_dram_parameter("bqk", [HPC, P], fp32, isOutput=False)
    bvr = nc.declare_dram_parameter("bvr", [P, HPC, P], fp32, isOutput=False)
    wo = nc.declare_dram_parameter("wo", [HPC, P, D], fp32, isOutput=False)
    mb = nc.declare_dram_parameter("mb", [B + 1, S], fp32, isOutput=False)
    oml = nc.declare_dram_parameter("oml", [P, 1], fp32, isOutput=False)
    a_out = nc.declare_dram_parameter("a_out", [HPC, B, S, S], fp32, isOutput=True)
    res_out = nc.declare_dram_parameter("res", [B, S, D], fp32, isOutput=True)

    from contextlib import ExitStack

    with ExitStack() as es:
        tc = es.enter_context(tile.TileContext(nc))
        const = es.enter_context(tc.tile_pool(name="const", bufs=1))
        xt_pool = es.enter_context(tc.tile_pool(name="xt", bufs=1))
        qt_pool = es.enter_context(tc.tile_pool(name="qt", bufs=1))
        kt_pool = es.enter_context(tc.tile_pool(name="kt", bufs=1))
        v_pool = es.enter_context(tc.tile_pool(name="vp", bufs=2))
        vt_pool = es.enter_context(tc.tile_pool(name="vt", bufs=2))
        p_pool = es.enter_context(tc.tile_pool(name="pp", bufs=2))
        pt_pool = es.enter_context(tc.tile_pool(name="pt", bufs=3))
        a_pool = es.enter_context(tc.tile_pool(name="aa", bufs=2))
        av_pool = es.enter_context(tc.tile_pool(name="av", bufs=4))
        ln_pool = es.enter_context(tc.tile_pool(name="ln", bufs=4))
        ot_pool = es.enter_context(tc.tile_pool(name="ot", bufs=2))
        r_pool = es.enter_context(tc.tile_pool(name="rr", bufs=2))
        acc_pool = es.enter_context(tc.tile_pool(name="acc", bufs=2))
        ps_sn = es.enter_context(tc.tile_pool(name="ps_sn", bufs=1, space="PSUM"))
        ps_st = es.enter_context(tc.tile_pool(name="ps_st", bufs=2, space="PSUM"))
        ps_av = es.enter_context(tc.tile_pool(name="ps_av", bufs=2, space="PSUM"))
        ps_tr = es.enter_context(tc.tile_pool(name="ps_tr", bufs=2, space="PSUM"))
        if True:
            fr = mybir.dt.float32r
            ident = const.tile([P, P], fp32)
            make_identity(nc, ident)
            wqk_sb = const.tile([P, HPC, KC, P], fr)
            nc.gpsimd.dma_start(
                out=wqk_sb,
                in_=wqk[:].rearrange("j (k p) m -> p j k m", p=P).bitcast(fr))
            wv_sb = const.tile([P, HPC, KC, P], fr)
            nc.gpsimd.dma_start(
                out=wv_sb,
                in_=wv[:].rearrange("j (k p) m -> p j k m", p=P).bitcast(fr))
            wo_sb = const.tile([P, HPC, D], fr)
            nc.gpsimd.dma_start(
                out=wo_sb, in_=wo[:].rearrange("j p m -> p j m").bitcast(fr))
            bqk_sb = const.tile([P, HPC], fp32)
            nc.gpsimd.dma_start(out=bqk_sb, in_=bqk[:].rearrange("j p -> p j"))
            oml_sb = const.tile([P, 1], fp32)
            nc.gpsimd.dma_start(out=oml_sb, in_=oml[:])
            eps_sb = const.tile([P, 1], fp32)
            nc.vector.memset(eps_sb, LN_EPS)
            bvr_sb = const.tile([P, HPC, P], fp32)
            nc.gpsimd.dma_start(out=bvr_sb, in_=bvr[:])
            bvo_sb = const.tile([P, HPC, P], fp32)
            # (1-lambda)*bv, replicated on all partitions
            nc.vector.tensor_scalar_mul(bvo_sb, bvr_sb, oml_sb)

            for b in range(B):
                xt = xt_pool.tile([P, KC, S], fr, tag="xt")
                nc.sync.dma_start(
                    out=xt, in_=xT[b].rearrange("(k p) s -> p k s", p=P).bitcast(fr))
                ots = []
                for j in range(HPC):
                    # ---- QK projection: psum rows 0:64 = Q-scaled, 64:128 = K
                    qt = qt_pool.tile([P, S], fr, tag="qt")
                    kt = kt_pool.tile([P, S], fr, tag="kt")
                    nc.sync.dma_start(out=qt[64:65, :], in_=mb[B:B + 1, :].bitcast(fr))
                    nc.sync.dma_start(out=kt[64:65, :], in_=mb[b:b + 1, :].bitcast(fr))
                    for n in range(S // 512):
                        ps = ps_tr.tile([P, 512], fp32, tag="ptr")
                        for k in range(KC):
                            nc.tensor.matmul(
                                ps, wqk_sb[:, j, k, :],
                                xt[:, k, n * 512:(n + 1) * 512],
                                start=(k == 0), stop=(k == KC - 1))
                        nc.vector.tensor_scalar_add(
                            qt[0:64, n * 512:(n + 1) * 512], ps[0:64, :],
                            bqk_sb[0:64, j:j + 1])
                        nc.vector.tensor_scalar_add(
                            kt[0:64, n * 512:(n + 1) * 512], ps[64:128, :],
                            bqk_sb[64:128, j:j + 1])
                    # ---- V projection (as V^T, then transpose to [t, v]) ----
                    vsb = v_pool.tile([P, TC, P], fr, tag="v")
                    for n in range(S // 512):
                        psv = ps_tr.tile([P, 512], fp32, tag="ptr")
                        for k in range(KC):
                            nc.tensor.matmul(
                                psv, wv_sb[:, j, k, :],
                                xt[:, k, n * 512:(n + 1) * 512],
                                start=(k == 0), stop=(k == KC - 1))
                        vts = vt_pool.tile([P, 512], fp32, tag="vt")
                        nc.vector.tensor_copy(vts, psv)
                        pst = ps_tr.tile([P, 512], fp32, tag="ptr")
                        for c4 in range(4):
                            nc.tensor.transpose(
                                pst[:, c4 * P:(c4 + 1) * P],
                                vts[:, c4 * P:(c4 + 1) * P], ident)
                        nc.vector.tensor_copy(vsb[:, n * 4:(n + 1) * 4, :], pst)
                    # ---- attention: phases interleaved per 512-wide sq block
                    cv_all = acc_pool.tile([P, NT], fp32, tag="cva")
                    mv_all = acc_pool.tile([P, NT, 2], fp32, tag="mva")
                    av_all = acc_pool.tile([P, NT, P], fp32, tag="ava")
                    ot = ot_pool.tile([P, NT, P], fr, tag="ot")
                    for beta in range(4):
                        sq0 = beta * 512
                        # phase A: natural scores -> exp(+rowsum) -> A
                        for n in range(beta * 4, beta * 4 + 4):
                            pex = p_pool.tile([P, S], fp32, tag="pp")
                            lh = ln_pool.tile([P, 2], fp32, tag="lh")
                            for h2 in range(2):
                                psn = ps_sn.tile([P, 1024], fp32, tag="sn")
                                for c in range(2):
                                    nc.tensor.matmul(
                                        psn[:, c * 512:(c + 1) * 512],
                                        qt[0:65, n * P:(n + 1) * P],
                                        kt[0:65, h2 * 1024 + c * 512:
                                           h2 * 1024 + (c + 1) * 512],
                                        start=True, stop=True)
                                nc.scalar.activation(
                                    pex[:, h2 * 1024:(h2 + 1) * 1024], psn,
                                    AF.Exp, accum_out=lh[:, h2:h2 + 1])
                            lsum = ln_pool.tile([P, 1], fp32, tag="ls")
                            nc.vector.tensor_add(lsum, lh[:, 0:1], lh[:, 1:2])
                            linv = ln_pool.tile([P, 1], fp32, tag="li")
                            nc.vector.reciprocal(linv, lsum)
                            nc.vector.tensor_mul(
                                cv_all[:, n:n + 1], linv, oml_sb)
                            for h2 in range(2):
                                a_sb = a_pool.tile([P, 1024], fp32, tag="aa")
                                nc.vector.tensor_scalar_mul(
                                    a_sb, pex[:, h2 * 1024:(h2 + 1) * 1024],
                                    cv_all[:, n:n + 1])
                                nc.sync.dma_start(
                                    out=a_out[j, b, n * P:(n + 1) * P,
                                              h2 * 1024:(h2 + 1) * 1024],
                                    in_=a_sb)
                        # phase B: transposed scores -> exp -> AV^T
                        ps_o = ps_av.tile([P, 512], fp32, tag="av")
                        for c in range(TC):
                            ps_t = ps_st.tile([P, 512], fp32, tag="st")
                            nc.tensor.matmul(
                                ps_t, kt[0:65, c * P:(c + 1) * P],
                                qt[0:65, sq0:sq0 + 512],
                                start=True, stop=True)
                            ptc = pt_pool.tile([P, 512], fr, tag="pt")
                            nc.scalar.activation(ptc, ps_t, AF.Exp)
                            nc.tensor.matmul(
                                ps_o, vsb[:, c, :], ptc,
                                start=(c == 0), stop=(c == TC - 1))
                        avt = vt_pool.tile([P, 512], fp32, tag="avt")
                        nc.vector.tensor_copy(avt, ps_o)
                        for q4 in range(4):
                            n = beta * 4 + q4
                            pstq = ps_tr.tile([P, 512], fp32, tag="ptr")
                            nc.tensor.transpose(
                                pstq[:, 0:P], avt[:, q4 * P:(q4 + 1) * P],
                                ident)
                            av = av_all[:, n, :]
                            nc.vector.tensor_scalar_mul(
                                av, pstq[:, 0:P], cv_all[:, n:n + 1])
                            nc.vector.tensor_add(av, av, bvo_sb[:, j, :])
                            stats = ln_pool.tile([P, 6], fp32, tag="st6")
                            nc.vector.bn_stats(stats, av)
                            nc.vector.bn_aggr(mv_all[:, n, :], stats)
                    # ---- batched LN rstd, apply, transpose to ot;
                    # last head also drives WO per tile so the batch tail
                    # pipelines instead of serializing ----
                    rstd = ln_pool.tile([P, NT], fp32, tag="rsd")
                    nc.scalar.activation(
                        rstd, mv_all[:, :, 1], AF.Sqrt, bias=eps_sb)
                    nc.vector.reciprocal(rstd, rstd)
                    for n in range(NT):
                        lno = av_pool.tile([P, P], fp32, tag="lno")
                        nc.vector.tensor_scalar(
                            lno, av_all[:, n, :], mv_all[:, n, 0:1],
                            rstd[:, n:n + 1],
                            op0=ALU.subtract, op1=ALU.mult)
                        psl = ps_tr.tile([P, 512], fp32, tag="ptr")
                        nc.tensor.transpose(psl[:, 0:P], lno, ident)
                        nc.vector.tensor_copy(ot[:, n, :], psl[:, 0:P])
                        if j == HPC - 1:
                            for half in range(2):
                                psw = ps_tr.tile([P, 512], fp32, tag="ptr")
                                for jj in range(HPC):
                                    nc.tensor.matmul(
                                        psw, ots[jj][:, n, :] if jj < j
                                        else ot[:, n, :],
                                        wo_sb[:, jj,
                                              half * 512:(half + 1) * 512],
                                        start=(jj == 0), stop=(jj == HPC - 1))
                                rsb = r_pool.tile([P, 512], fp32, tag="rr")
                                nc.vector.tensor_copy(rsb, psw)
                                nc.sync.dma_start(
                                    out=res_out[b, n * P:(n + 1) * P,
                                                half * 512:(half + 1) * 512],
                                    in_=rsb)
                    ots.append(ot)

    nc.finalize()
    return nc


def _prep_inputs(x, lambda_current, mask, wq, bq, wk, bk, wv, bv, wo):
    """Host-side sharding/marshaling. Returns (shared, percore) dicts."""
    f = np.float32
    xT = np.ascontiguousarray(np.transpose(np.asarray(x, f), (0, 2, 1)))
    mbias = (np.asarray(mask, f) - 1.0) * np.float32(1e30)
    mbias = np.concatenate([mbias, np.ones((1, S), f)], axis=0)
    omlv = np.full((P, 1), 1.0 - float(lambda_current), f)
    shared = {"xT": xT, "mb": mbias, "oml": omlv}
    percore = []
    for i in range(NCORES):
        h0 = i * HPC
        wqk_i = np.stack([
            np.concatenate([wq[h] * SCALE, wk[h]], axis=1)
            for h in range(h0, h0 + HPC)]).astype(f)
        bqk_i = np.stack([
            np.concatenate([bq[h] * SCALE, bk[h]])
            for h in range(h0, h0 + HPC)]).astype(f)
        wv_i = np.ascontiguousarray(wv[h0:h0 + HPC], f)
        bvr_i = np.broadcast_to(
            bv[h0:h0 + HPC][None, :, :], (P, HPC, P)).astype(f)
        wo_i = np.ascontiguousarray(
            wo[h0 * P:(h0 + HPC) * P].reshape(HPC, P, D), f)
        percore.append({
            "wqk": wqk_i, "bqk": bqk_i, "wv": wv_i, "bvr": bvr_i, "wo": wo_i})
    return shared, percore


def kernel(x, lambda_current, mask, wq, bq, wk, bk, wv, bv, wo, bo, ln_g, ln_b):
    _setup_paths()
    _setup_ntff_hook()
    from concourse.bass_utils import run_bass_kernel_spmd

    f = np.float32
    x = np.asarray(x, f)
    wq = np.asarray(wq, f); bq = np.asarray(bq, f)
    wk = np.asarray(wk, f); bk = np.asarray(bk, f)
    wv_a = np.asarray(wv, f); bv_a = np.asarray(bv, f)
    wo_a = np.asarray(wo, f); bo_a = np.asarray(bo, f)
    ln_g = np.asarray(ln_g, f); ln_b = np.asarray(ln_b, f)

    # Fold LN gamma (and 1-LAMBDA_INIT) into WO; beta becomes an output offset.
    gfac = (1.0 - LAMBDA_INIT) * ln_g            # [128]
    gtile = np.tile(gfac, H)                     # [2048]
    wo_eff = (wo_a * gtile[:, None]).astype(f)
    btile = np.tile((1.0 - LAMBDA_INIT) * ln_b, H)
    out_off = (btile @ wo_a + bo_a).astype(f)    # [1024]

    shared, percore = _prep_inputs(
        x, lambda_current, mask, wq, bq, wk, bk, wv_a, bv_a, wo_eff)

    if "nc" not in _CACHE:
        _CACHE["nc"] = _build_program()
    nc = _CACHE["nc"]

    in_maps = [dict(shared, **pc) for pc in percore]
    trace = bool(os.environ.get("BASS_KERNEL_TRACE"))
    r = run_bass_kernel_spmd(nc, in_maps, list(range(NCORES)), trace=trace)
    _CACHE["last_exec_time_ns"] = r.exec_time_ns

    A = np.concatenate([r.results[i]["a_out"] for i in range(NCORES)], axis=0)
    res = np.zeros((B, S, D), f)
    for i in range(NCORES):
        res += r.results[i]["res"]
    res += out_off[None, None, :]
    return (res, A)
